# revision 16
# baseline (speedup 1.0000x reference)
"""Trainium2 Bass kernel for nn_Mnist_lmdSplineKAN — fp8 DoubleRow rewrite.

Sharding: data-parallel over batch, 8 cores x 128 rows; params replicated.

Math (per core, I=784, H=10, O=64, grid [0,1] / 5 intervals, cubic):
  t = floor(5x) -> one-hot masks; u = frac(5x); local cubics
  p = [u^3, p2, p1, w^3] (w=1-u), each MEAN-CENTERED over U[0,1).
  26 fp8 feature planes: 20 products M_t*pc_s (bitwise-AND of 0xFF byte
  masks with fp8 poly bytes == exact mask-multiply), 5 exact mask planes
  (carry the centering constants), 1 centered silu plane.
  y[b,(h,o)] = sum_{i,plane} z[b,i,plane] * W8[(i,plane),(h,o)]  via fp8
  DoubleRow matmuls (2x128 contraction rows per instr, 0.5 cyc/row),
  PSUM fp32.  Tail: tanh -> transpose -> blockdiag W1 -> tanh -> W2.

Weights are scaled by SW=16 on host (fp8 normal range); the tanh after
PSUM applies scale 1/16.  Weight pairs stream from HBM over all 4 DMA
queues (sync/scalar/vector/pool) in consumption order; PE is kept warm
by dummy matmuls so real ones run at full clock.
"""
import sys, types
import numpy as np

B, I, O, H = 1024, 784, 64, 10
NC = 8
BC = B // NC          # 128 batch rows per core
CH = 7                # 6 full 128-row i-chunks + 1 of 16
PLAST = 16
HO = H * O            # 640
D2 = H * 32           # 320
NH = 2
NPL = 26              # 20 product planes + 5 mask planes + silu
SW = 16.0
EP = [0.25, 2.75, 2.75, 0.25]      # E[p_s] over u~U[0,1)
ES = 0.32944265                    # E[silu(x)], x~U[0,1)
NDUM = 14

# consumption order of planes (by feature-completion time)
P_ORDER = [25, 20, 21, 22, 23, 24] + list(range(20))
FULL_PAIRS = [(p, cp) for p in P_ORDER for cp in range(3)]   # 78
TAIL_PAIRS = [(2 * q, 2 * q + 1) for q in range(13)]  # adjacent plane idx


def _unit_key(p, cp):
    """Dedup'd weight unit for plane p, chunk-pair cp: product planes with
    the same j share one weight block."""
    if p == 25:
        return ("silu", cp)
    if p >= 20:
        return ("M", p - 20, cp)
    t, s = p // 4, p % 4
    return ("j", t + 3 - s, cp)


UNITS = []          # first-use order
for _p in P_ORDER:
    for _cp in range(3):
        _k = _unit_key(_p, _cp)
        if _k not in UNITS:
            UNITS.append(_k)        # 42 units

QNAMES = ["sync", "scalar", "pool"]
QBATCH = {"sync": 4, "scalar": 8, "pool": 8}
QSTART = {"sync": 2000.0, "scalar": 800.0, "pool": 900.0}
PAIR_NS = 494.0


def _schedule_units():
    qt = dict(QSTART)
    cur = {q: [] for q in QNAMES}
    issues = {q: [] for q in QNAMES}
    for k in range(len(UNITS)):
        q = min(QNAMES, key=lambda n: qt[n])
        qt[q] += PAIR_NS
        cur[q].append(k)
        if len(cur[q]) == QBATCH[q]:
            issues[q].append(cur[q]); cur[q] = []
    for q in QNAMES:
        if cur[q]:
            issues[q].append(cur[q])
    return issues


QISSUES = _schedule_units()
QUNITS = {q: [k for blk in QISSUES[q] for k in blk] for q in QNAMES}
UNIT_LOC = {}
for _q in QNAMES:
    for _pos, _k in enumerate(QUNITS[_q]):
        UNIT_LOC[UNITS[_k]] = (_q, _pos)


def _install_ntff_hook():
    if "antenv.axon_hooks" in sys.modules:
        return
    try:
        import antenv
        mod = types.ModuleType("antenv.axon_hooks")
        _h = [None]
        mod.set_axon_ntff_profile_hook = lambda h: _h.__setitem__(0, h)
        mod.get_axon_ntff_profile_hook = lambda: _h[0]
        sys.modules["antenv.axon_hooks"] = mod
        antenv.axon_hooks = mod
        from trn_agent_boot.trn_boot import _ntff_profile_via_ctypes
        h = _ntff_profile_via_ctypes("/opt/axon/libaxon_pjrt.so")
        if h is not None:
            mod.set_axon_ntff_profile_hook(h)
    except Exception:
        pass


_CACHE = {}


def _build():
    if "nc" in _CACHE:
        return _CACHE["nc"]
    import concourse.bacc as bacc
    import concourse.bass as bass
    import concourse.tile as tile
    from concourse import mybir
    from contextlib import ExitStack

    f32, f16 = mybir.dt.float32, mybir.dt.float16
    i16, i8 = mybir.dt.int16, mybir.dt.int8
    u32 = mybir.dt.uint32
    fp8 = mybir.dt.float8e4
    ALU = mybir.AluOpType
    AF = mybir.ActivationFunctionType
    DR = mybir.MatmulPerfMode.DoubleRow

    nc = bacc.Bacc("TRN2", target_bir_lowering=False, debug=False)
    x_d = nc.dram_tensor("x", (128, CH, BC), f32, kind="ExternalInput").ap()
    wq_d = {}
    for q in QNAMES:
        nq = len(QUNITS[q])
        wq_d[q] = nc.dram_tensor(f"w_{q}", (128 * nq * 1280,), fp8,
                                 kind="ExternalInput").ap()
    wt_d = nc.dram_tensor("wt", (PLAST * 13 * 1280,), fp8,
                          kind="ExternalInput").ap()
    c16_d = nc.dram_tensor("c16", (128, 5 * D2 + 128), f16,
                           kind="ExternalInput").ap()
    c32_d = nc.dram_tensor("c32", (128, D2 + H), f32,
                           kind="ExternalInput").ap()
    b1_d = nc.dram_tensor("b1", (1, D2), f16, kind="ExternalInput").ap()
    out_d = nc.dram_tensor("out", (BC, H), f32, kind="ExternalOutput").ap()

    with tile.TileContext(nc) as tc, ExitStack() as ctx:
        sb = ctx.enter_context(tc.tile_pool(name="sb", bufs=1))
        ps = ctx.enter_context(tc.tile_pool(name="ps", bufs=1, space="PSUM"))

        # ---------------- DMA section ----------------
        xt = sb.tile([128, CH, BC], f32, tag="xt")
        nc.sync.dma_start(xt[:], x_d)

        wq = {}
        for q in QNAMES:
            nq = len(QUNITS[q])
            wq[q] = sb.tile([128, nq, 2, HO], fp8, tag=f"wq_{q}",
                            name=f"wq_{q}")
        eng = {"sync": nc.sync, "scalar": nc.scalar, "pool": nc.gpsimd}

        def emit_wdma(q, bi):
            blk = QISSUES[q][bi]
            a = QUNITS[q].index(blk[0])
            n = len(blk)
            nq = len(QUNITS[q])
            src = bass.AP(tensor=wq_d[q].tensor, offset=a * 1280,
                          ap=[[nq * 1280, 128], [1, n * 1280]])
            dst = wq[q][:, a:a + n, :, :].rearrange("p n m o -> p (n m o)")
            eng[q].dma_start(dst, src)

        # pool queue issues go first (engine idle until x lands)
        for bi in range(len(QISSUES["pool"])):
            emit_wdma("pool", bi)
        for bi in range(len(QISSUES["sync"])):
            emit_wdma("sync", bi)
        emit_wdma("scalar", 0)

        # small consts on sync (idle engine)
        c16 = sb.tile([128, 5 * D2 + 128], f16, tag="c16")
        nc.sync.dma_start(c16[:], c16_d)
        w1t = c16[:, 0:5 * D2].rearrange("p (k d) -> p k d", d=D2)
        idt = c16[:, 5 * D2:]
        c32 = sb.tile([128, D2 + H], f32, tag="c32")
        nc.sync.dma_start(c32[:], c32_d)
        w2b = c32[:, 0:D2]
        b2b = c32[:, D2:]
        b1r = sb.tile([1, D2], f16, tag="b1r")
        nc.sync.dma_start(b1r[:], b1_d)

        # ---------------- small consts / dummies ----------------
        ones = sb.tile([1, 128], f16, tag="ones")
        nc.vector.memset(ones[:], 1.0)
        dum8 = sb.tile([128, 2, D2], fp8, tag="dum8")
        nc.vector.memset(dum8[:].rearrange("p m o -> p (m o)"), 1.0)

        # ---------------- features ----------------
        x = xt[:].rearrange("p c b -> p (c b)")
        NF = CH * BC

        def T(tag, dt=f16):
            return sb.tile([128, NF], dt, tag=tag, name=tag)

        ti16 = T("ti16", i16)
        nc.vector.tensor_scalar(ti16[:], x, 5.0, -0.5, op0=ALU.mult,
                                op1=ALU.add)
        u = T("u")
        nc.vector.scalar_tensor_tensor(u[:], x, 5.0, ti16[:],
                                       op0=ALU.mult, op1=ALU.subtract)
        w = T("w")
        nc.vector.tensor_scalar(w[:], u[:], -1.0, 1.0, op0=ALU.mult,
                                op1=ALU.add)
        v = T("v")
        nc.vector.tensor_scalar(v[:], u[:], -2.0, None, op0=ALU.add)
        vw = T("vw")
        nc.vector.tensor_scalar(vw[:], u[:], -1.0, -1.0, op0=ALU.mult,
                                op1=ALU.add)

        silu16 = T("silu16")
        nc.scalar.activation(silu16[:], x, AF.Silu)
        u2 = T("u2")
        nc.scalar.activation(u2[:], u[:], AF.Square)
        w2 = T("w2")
        nc.scalar.activation(w2[:], u[:], AF.Square, bias=1.0, scale=-1.0)

        # byte masks 0xFF/0x00 (int8 -1/0) on pool
        MB = sb.tile([128, 5, NF], i8, tag="MB")
        for t in range(5):
            nc.gpsimd.tensor_scalar(MB[:, t, :], ti16[:], t, -1,
                                    op0=ALU.is_equal, op1=ALU.mult)

        m1 = T("m1")
        nc.gpsimd.tensor_tensor(m1[:], u2[:], v[:], op=ALU.mult)
        m2 = T("m2")
        nc.gpsimd.tensor_tensor(m2[:], w2[:], vw[:], op=ALU.mult)

        PRc = sb.tile([128, 4, NF], fp8, tag="PRc")
        u3 = T("u3")
        nc.vector.tensor_tensor(u3[:], u2[:], u[:], op=ALU.mult)
        w3 = T("w3")
        nc.vector.tensor_tensor(w3[:], w2[:], w[:], op=ALU.mult)
        nc.vector.tensor_scalar(PRc[:, 0, :], u3[:], 1.0, -EP[0],
                                op0=ALU.mult, op1=ALU.add)
        nc.vector.tensor_scalar(PRc[:, 3, :], w3[:], 1.0, -EP[3],
                                op0=ALU.mult, op1=ALU.add)
        nc.scalar.activation(PRc[:, 1, :], m2[:], AF.Copy, bias=4.0 - EP[1],
                             scale=3.0)
        nc.scalar.activation(PRc[:, 2, :], m1[:], AF.Copy, bias=4.0 - EP[2],
                             scale=3.0)

        # feature plane store
        Z = sb.tile([128, NPL, CH, BC], fp8, tag="Z")

        def zplane(p):
            return Z[:, p, :, :].rearrange("p c b -> p (c b)")

        # silu plane (centered) on pool
        nc.gpsimd.tensor_scalar(zplane(25), silu16[:], 1.0, -ES,
                                op0=ALU.mult, op1=ALU.add)
        # mask planes -> fp8 1.0/0.0 (pool)
        for t in range(5):
            nc.gpsimd.tensor_scalar(zplane(20 + t), ti16[:], t, None,
                                    op0=ALU.is_equal)
        # product planes: byte-mask AND poly bytes, as uint32 on DVE
        # (bitwise ops are DVE + 32-bit only)
        prv = PRc[:].rearrange("p s f -> p (s f)").bitcast(u32) \
            .rearrange("p (s f) -> p s f", s=4)
        for t in range(5):
            mbv = MB[:, t, :].bitcast(u32)
            mbb = bass.AP(tensor=mbv.tensor, offset=mbv.offset,
                          ap=[mbv.ap[0], [0, 4], mbv.ap[1]])
            dst = Z[:, 4 * t:4 * t + 4, :, :] \
                .rearrange("p s c b -> p (s c b)").bitcast(u32) \
                .rearrange("p (s f) -> p s f", s=4)
            nc.vector.tensor_tensor(dst, mbb, prv, op=ALU.bitwise_and)

        # remaining scalar-queue weight blocks
        for bi in range(1, len(QISSUES["scalar"])):
            emit_wdma("scalar", bi)

        # tail weights + their DMA (pool, consumed last)
        wt = sb.tile([PLAST, 13, 2, HO], fp8, tag="wt")
        src = bass.AP(tensor=wt_d.tensor, offset=0,
                      ap=[[13 * 1280, PLAST], [1, 13 * 1280]])
        nc.gpsimd.dma_start(wt[:].rearrange("p n m o -> p (n m o)"), src)

        # ---------------- main matmuls ----------------
        psum = [ps.tile([128, D2], f32, tag=f"y{nh}", name=f"y{nh}")
                for nh in range(NH)]
        # PE warm-up: keep the tensor engine busy from t=0 so the p-state
        # ramp completes before real matmuls; each dummy is a closed group.
        for _ in range(NDUM):
            nc.tensor.matmul(psum[0][:], dum8[:, :, 0:128], dum8[:],
                             start=True, stop=True, perf_mode=DR)
        first = [True, True]
        for k, (p, cp) in enumerate(FULL_PAIRS):
            q, pos = UNIT_LOC[_unit_key(p, cp)]
            lhs = Z[:, p, 2 * cp:2 * cp + 2, :]
            for nh in range(NH):
                nc.tensor.matmul(psum[nh][:], lhs,
                                 wq[q][:, pos, :, nh * D2:(nh + 1) * D2],
                                 start=first[nh], stop=False, perf_mode=DR)
                first[nh] = False
        for qi in range(13):
            pa, pb = TAIL_PAIRS[qi]
            # two plane-tails (chunk 6) as one DoubleRow pair
            lhs = Z[0:PLAST, pa:pa + 2, 6, :]
            for nh in range(NH):
                nc.tensor.matmul(psum[nh][:], lhs,
                                 wt[:, qi, :, nh * D2:(nh + 1) * D2],
                                 start=False, stop=(qi == 12),
                                 perf_mode=DR)

        # ---------------- tail MLP ----------------
        h1 = sb.tile([128, HO], f16, tag="h1")
        SEG = [(0, 0, 128), (0, 128, 256), (0, 256, 320), (1, 320, 384),
               (1, 384, 512), (1, 512, 640)]

        def tanh_seg(k):
            nh, s0, s1 = SEG[k]
            nc.scalar.activation(h1[:, s0:s1],
                                 psum[nh][:, s0 - nh * D2:s1 - nh * D2],
                                 AF.Tanh, scale=1.0 / SW)

        h1t = []

        def tr(k):
            pt = ps.tile([128, 128], f16, tag=f"pt{k}", name=f"pt{k}")
            nc.tensor.transpose(pt[:], h1[:, k * 128:(k + 1) * 128], idt)
            st = sb.tile([128, 128], f16, tag=f"h1t{k}", name=f"h1t{k}")
            nc.vector.tensor_copy(st[:], pt[:])
            h1t.append(st)

        tanh_seg(0); tr(0)
        tanh_seg(1); tr(1)
        tanh_seg(2); tanh_seg(3); tr(2)
        tanh_seg(4); tr(3)
        tanh_seg(5); tr(4)

        ps2 = ps.tile([128, D2], f32, tag="ps2")
        for k in range(5):
            nc.tensor.matmul(ps2[:], h1t[k][:], w1t[:, k, :],
                             start=(k == 0), stop=False)
        nc.tensor.matmul(ps2[:], ones[:], b1r[:], start=False, stop=True)
        h2 = sb.tile([128, D2], f32, tag="h2")
        nc.scalar.activation(h2[:], ps2[:], AF.Tanh)
        prod = sb.tile([128, D2], f32, tag="prod")
        nc.vector.tensor_tensor(prod[:], h2[:], w2b, op=ALU.mult)
        red = sb.tile([128, H], f32, tag="red")
        nc.vector.tensor_reduce(red[:], prod[:].rearrange(
            "p (h d) -> p h d", d=32), axis=mybir.AxisListType.X, op=ALU.add)
        lg = sb.tile([128, H], f32, tag="lg")
        nc.vector.tensor_tensor(lg[:], red[:], b2b, op=ALU.add)
        nc.sync.dma_start(out_d, lg[:])

    nc.compile()
    _CACHE["nc"] = nc
    return nc


def _plane_weights(coef, scale_base, scale_sp, lmd):
    """Return W8[plane][i, ho] fp32 (pre-SW), plane order 0..25."""
    eff = (coef * scale_sp[..., None] * lmd[:, :, None, None] / 6.0)
    effT = eff.transpose(1, 3, 0, 2).reshape(I, 8, HO)   # (i, j, ho)
    sblf = (scale_base * lmd[:, :, None]).transpose(1, 0, 2).reshape(I, HO)
    Wp = np.zeros((NPL, I, HO), np.float32)
    for t in range(5):
        for s in range(4):
            j = t + 3 - s
            if 0 <= j < 8:
                Wp[4 * t + s] = effT[:, j]
    for t in range(5):
        acc = np.zeros((I, HO), np.float32)
        for s in range(4):
            acc += EP[s] * Wp[4 * t + s]
        Wp[20 + t] = acc + ES * sblf
    Wp[25] = sblf
    return Wp


def _prep_inputs(x, coef, scale_base, scale_sp, lmd, W1, b1, W2, b2):
    import ml_dtypes
    e4 = ml_dtypes.float8_e4m3
    xf = np.asarray(x, np.float32).reshape(B, I)
    Wp = _plane_weights(np.asarray(coef, np.float64),
                        np.asarray(scale_base, np.float64),
                        np.asarray(scale_sp, np.float64),
                        np.asarray(lmd, np.float64))
    W8 = (Wp * SW).astype(e4)                     # (NPL, I, HO) fp8

    def unit_plane(key):
        kind = key[0]
        if kind == "silu":
            return 25
        if kind == "M":
            return 20 + key[1]
        j = key[1]                      # any plane with t+3-s == j
        for p in range(20):
            if p // 4 + 3 - p % 4 == j:
                return p
        raise KeyError(key)

    # dedup'd unit buffers per queue: [part, unitpos, m, ho]
    wbufs = {}
    for q in QNAMES:
        nq = len(QUNITS[q])
        buf = np.zeros((128, nq, 2, HO), e4)
        for pos, k in enumerate(QUNITS[q]):
            key = UNITS[k]
            p, cp = unit_plane(key), key[-1]
            for m in range(2):
                i0 = (2 * cp + m) * 128
                buf[:, pos, m, :] = W8[p, i0:i0 + 128, :]
        wbufs[q] = np.ascontiguousarray(buf).reshape(-1)
    wtail = np.zeros((PLAST, 13, 2, HO), e4)
    for qi, (pa, pb) in enumerate(TAIL_PAIRS):
        wtail[:, qi, 0, :] = W8[pa, 768:I, :]
        wtail[:, qi, 1, :] = W8[pb, 768:I, :]
    wtail = np.ascontiguousarray(wtail).reshape(-1)

    W1 = np.asarray(W1, np.float64)
    w1bd = np.zeros((HO, D2))
    for h in range(H):
        w1bd[h * O:(h + 1) * O, h * 32:(h + 1) * 32] = W1[h]
    w1dev = np.ascontiguousarray(
        w1bd.reshape(5, 128, D2).transpose(1, 0, 2)).astype(np.float16)
    c16 = np.concatenate([w1dev.reshape(128, 5 * D2),
                          np.eye(128, dtype=np.float16)], 1).astype(np.float16)
    b1c = np.asarray(b1, np.float16).reshape(1, D2).copy()
    c32 = np.ascontiguousarray(np.concatenate([
        np.broadcast_to(np.asarray(W2, np.float32).reshape(D2), (128, D2)),
        np.broadcast_to(np.asarray(b2, np.float32).reshape(H), (128, H))],
        1).astype(np.float32))

    in_maps = []
    for core in range(NC):
        xs = xf[core * BC:(core + 1) * BC].T              # (784,128)
        xdev = np.zeros((128, CH, BC), np.float32)
        for c in range(CH):
            rows = xs[c * 128:min((c + 1) * 128, I)]
            xdev[0:rows.shape[0], c, :] = rows
        m = {"x": xdev, "wt": wtail, "c16": c16, "b1": b1c, "c32": c32}
        for q in QNAMES:
            m[f"w_{q}"] = wbufs[q]
        in_maps.append(m)
    return in_maps


def run(inputs, trace=False, tmpdir=None):
    _install_ntff_hook()
    from concourse.bass_utils import run_bass_kernel_spmd
    nc = _build()
    in_maps = _prep_inputs(**inputs)
    res = run_bass_kernel_spmd(nc, in_maps, core_ids=list(range(NC)),
                               trace=trace, tmpdir=tmpdir)
    out = np.concatenate([r["out"] for r in res.results], 0)
    return out.astype(np.float32), res


def kernel(**inputs):
    out, _ = run(inputs)
    return out


# revision 22
# speedup vs baseline: 2.5647x; 2.5647x over previous
"""Trainium2 Bass kernel for nn_Mnist_lmdSplineKAN.

Sharding: data-parallel over batch, 8 cores x 128 rows. All params replicated.

Per-core math (I=784 inputs, H=10 heads, O=64, 8 B-spline basis fns, order 3,
5 uniform intervals on [0,1)):
  t = floor(5x) (int-round trick), u = 5x - t, one-hot masks m_t = (t == const)
  features[b,i,j] = sum_t m_t * p_{j-t}(u)  with p = 6x local cubic polys
  features[b,i,8] = silu(x[b,i])
  y[b,(h,o)] = sum_{i,j} features[b,i,j] * Wbig[(i,j),(h,o)]  (fp16 matmul;
               Wbig folds coef*scale_sp*lmd/6 and scale_base*lmd)
  h1 = tanh(y); h2 = tanh(h1 @ blockdiag(W1) + b1); logits = <h2,W2>_head + b2

I is tiled as 6 chunks of 128 (full partitions, FWL-eligible) + 1 of 16.
Weights stream as per-chunk piece-major contiguous DMAs on the SWDGE queue;
matmuls are emitted in a wavefront order matching weight-arrival (c) and
feature-completion (j) times so the PE FIFO never head-blocks.
"""
import sys, types
import numpy as np

B, I, O, H, NB = 1024, 784, 64, 10, 8
NC = 8
BC = B // NC      # 128
CH = 7            # 6 full 128-row chunks + 1 of 16
PLAST = 16
HO = H * O        # 640
D2 = H * 32       # 320
NH = 2


def _install_ntff_hook():
    if "antenv.axon_hooks" in sys.modules:
        return
    try:
        import antenv
        mod = types.ModuleType("antenv.axon_hooks")
        _h = [None]
        mod.set_axon_ntff_profile_hook = lambda h: _h.__setitem__(0, h)
        mod.get_axon_ntff_profile_hook = lambda: _h[0]
        sys.modules["antenv.axon_hooks"] = mod
        antenv.axon_hooks = mod
        from trn_agent_boot.trn_boot import _ntff_profile_via_ctypes
        h = _ntff_profile_via_ctypes("/opt/axon/libaxon_pjrt.so")
        if h is not None:
            mod.set_axon_ntff_profile_hook(h)
    except Exception:
        pass


_CACHE = {}


def _build():
    if "nc" in _CACHE:
        return _CACHE["nc"]
    import concourse.bacc as bacc
    import concourse.bass as bass
    import concourse.tile as tile
    from concourse import mybir
    from contextlib import ExitStack

    f32, f16, i32 = mybir.dt.float32, mybir.dt.float16, mybir.dt.int32
    ALU = mybir.AluOpType
    AF = mybir.ActivationFunctionType

    nc = bacc.Bacc("TRN2", target_bir_lowering=False, debug=False)
    x_d = nc.dram_tensor("x", (128, CH, BC), f32, kind="ExternalInput").ap()
    w_d = nc.dram_tensor("w", (I * (NB + 1) * HO,), f16,
                         kind="ExternalInput").ap()
    w1_d = nc.dram_tensor("w1", (128, 5 * D2 + 128), f16,
                          kind="ExternalInput").ap()
    b1_d = nc.dram_tensor("b1", (1, D2), f16, kind="ExternalInput").ap()
    w2_d = nc.dram_tensor("w2", (128, D2 + H), f32, kind="ExternalInput").ap()
    out_d = nc.dram_tensor("out", (BC, H), f32, kind="ExternalOutput").ap()

    with tile.TileContext(nc) as tc, ExitStack() as ctx:
        sb = ctx.enter_context(tc.tile_pool(name="sb", bufs=1))
        ps = ctx.enter_context(tc.tile_pool(name="ps", bufs=1, space="PSUM"))

        # ---- x split across both HWDGE queues: lands first ----
        xt = sb.tile([128, CH, BC], f32, tag="xt")
        nc.sync.dma_start(xt[:, 0:4, :], x_d[:, 0:4, :])
        nc.scalar.dma_start(xt[:, 4:CH, :], x_d[:, 4:CH, :])
        ones = sb.tile([1, 128], f16, tag="ones")
        nc.vector.memset(ones[:], 1.0)

        # ---- weights: piece-major contiguous pieces on the SWDGE queue in
        #      consumption order; last (16-row) chunk split by output half ----
        ROW = (NB + 1) * HO
        wg = []
        off = 0
        for c in range(6):
            t = sb.tile([128, NB + 1, HO], f16, tag=f"wg{c}", name=f"wg{c}")
            src = bass.AP(tensor=w_d.tensor, offset=off,
                          ap=[[ROW, 128], [1, ROW]])
            nc.gpsimd.dma_start(t[:], src)
            wg.append(t)
            off += 128 * ROW
        wg6 = []
        for nh in range(NH):
            t = sb.tile([PLAST, NB + 1, D2], f16, tag=f"wg6{nh}",
                        name=f"wg6{nh}")
            run = (NB + 1) * D2
            src = bass.AP(tensor=w_d.tensor, offset=off,
                          ap=[[run, PLAST], [1, run]])
            nc.gpsimd.dma_start(t[:], src)
            wg6.append(t)
            off += PLAST * run

        def wslice(c, j, nh):
            if c < 6:
                return wg[c][:, j, nh * D2:(nh + 1) * D2]
            return wg6[nh][:, j, :]

        # ---- tail consts, trailing on the SWDGE queue ----
        c16 = sb.tile([128, 5 * D2 + 128], f16, tag="c16")
        nc.gpsimd.dma_start(c16[:], w1_d)
        w1t = c16[:, 0:5 * D2].rearrange("p (k d) -> p k d", d=D2)
        idt = c16[:, 5 * D2:]
        c32 = sb.tile([128, D2 + H], f32, tag="c32")
        nc.gpsimd.dma_start(c32[:], w2_d)
        w2b = c32[:, 0:D2]
        b2b = c32[:, D2:]
        b1r = sb.tile([1, D2], f16, tag="b1r")
        nc.gpsimd.dma_start(b1r[:], b1_d)

        x = xt[:].rearrange("p c b -> p (c b)")

        def T(tag, dt=f16):
            return sb.tile([128, CH * BC], dt, tag=tag, name=tag)

        # ---- features tiles; silu first on ACT (only needs x) ----
        f_ = {}
        for j in range(NB):
            f_[j] = sb.tile([128, CH, BC], f16, tag=f"f{j}", name=f"f{j}")
        fs = sb.tile([128, CH, BC], f16, tag="f8")
        nc.scalar.activation(fs[:].rearrange("p c b -> p (c b)"), x, AF.Silu)
        f_[NB] = fs

        # ---- interval index t = floor(5x) via round(5x-0.5); u; masks ----
        ti = T("ti", i32)
        nc.vector.tensor_scalar(ti[:], x, 5.0, -0.5, op0=ALU.mult, op1=ALU.add)
        u = T("u", f32)
        nc.vector.scalar_tensor_tensor(u[:], x, 5.0, ti[:],
                                       op0=ALU.mult, op1=ALU.subtract)
        M = sb.tile([128, 5, CH * BC], f16, tag="M")
        for t in range(5):
            nc.vector.tensor_scalar(M[:, t, :], ti[:], t, None, op0=ALU.is_equal)

        # ---- local cubics (x6): p0=(1-u)^3, p1=(3u-6)u^2+4, p2=p1(1-u),
        #      p3=u^3;  ACT makes f16 operands, DVE multiplies at 2x ----
        u_h = T("u_h"); nc.scalar.activation(u_h[:], u[:], AF.Copy)
        u2h = T("u2h"); nc.scalar.activation(u2h[:], u[:], AF.Square)
        w_ = T("w_")
        nc.scalar.activation(w_[:], u[:], AF.Copy, bias=1.0, scale=-1.0)
        w2h = T("w2h"); nc.scalar.activation(w2h[:], w_[:], AF.Square)
        a_ = T("a_")
        nc.scalar.activation(a_[:], u[:], AF.Copy, bias=-6.0, scale=3.0)
        b_ = T("b_")
        nc.scalar.activation(b_[:], w_[:], AF.Copy, bias=-6.0, scale=3.0)
        PR = sb.tile([128, 4, CH * BC], f16, tag="PR")
        nc.vector.tensor_tensor(PR[:, 0, :], u2h[:], u_h[:], op=ALU.mult)   # p3
        nc.vector.tensor_tensor(PR[:, 3, :], w2h[:], w_[:], op=ALU.mult)    # p0
        p1pre = T("p1pre")
        nc.vector.tensor_tensor(p1pre[:], a_[:], u2h[:], op=ALU.mult)
        nc.scalar.activation(PR[:, 2, :], p1pre[:], AF.Copy, bias=4.0, scale=1.0)
        p2pre = T("p2pre")
        nc.vector.tensor_tensor(p2pre[:], b_[:], w2h[:], op=ALU.mult)
        nc.scalar.activation(PR[:, 1, :], p2pre[:], AF.Copy, bias=4.0, scale=1.0)

        psum = [ps.tile([128, D2], f32, tag=f"y{nh}", name=f"y{nh}")
                for nh in range(NH)]

        JORDER = (0, 7, 1, 6, 2, 5, 3, 4)
        tk = sb.tile([128, 4, CH * BC], f16, tag="tk")
        t2 = sb.tile([128, 2, CH * BC], f16, tag="t2")
        tmp = T("tmp")
        for j in JORDER:
            tlo = max(0, j - 3)
            k = min(4, j) - tlo + 1
            s0 = 3 - min(j, 3)
            out = f_[j][:].rearrange("p c b -> p (c b)")
            if k == 1:
                nc.vector.tensor_tensor(out, M[:, tlo, :], PR[:, s0, :],
                                        op=ALU.mult)
                continue
            nc.vector.tensor_tensor(tk[:, 0:k, :], M[:, tlo:tlo + k, :],
                                    PR[:, s0:s0 + k, :], op=ALU.mult)
            if k == 2:
                nc.vector.tensor_tensor(out, tk[:, 0, :], tk[:, 1, :], op=ALU.add)
            elif k == 3:
                nc.vector.tensor_tensor(tmp[:], tk[:, 0, :], tk[:, 1, :], op=ALU.add)
                nc.vector.tensor_tensor(out, tmp[:], tk[:, 2, :], op=ALU.add)
            else:
                nc.vector.tensor_tensor(t2[:], tk[:, 0:2, :], tk[:, 2:4, :],
                                        op=ALU.add)
                nc.vector.tensor_tensor(out, t2[:, 0, :], t2[:, 1, :], op=ALU.add)

        # ---- main matmuls in wavefront order ----
        # feature completion follows JORDER; cumulative DVE ops to finish j
        cumops = {}
        acc = 0
        for j in JORDER:
            acc += 2 * len([t for t in range(5) if 0 <= j - t <= 3]) - 1
            cumops[j] = acc

        cumm = {}
        acc = 0
        for j in JORDER:
            k = len([t for t in range(5) if 0 <= j - t <= 3])
            acc += k
            cumm[j] = (acc, k)

        RJ = {0: 4.0, 7: 4.6, 1: 6.5, 6: 8.2, 2: 10.8, 5: 13.5,
              3: 17.2, 4: 20.7, NB: -3.0}

        def ready(cj):
            c, j = cj
            return max(3.45 * c, RJ[j])
        order = sorted(((c, j) for c in range(CH) for j in range(NB + 1)),
                       key=lambda cj: (ready(cj), cj[0]))
        NTOT = CH * (NB + 1)
        for nmm, (c, j) in enumerate(order):
            lhs = f_[j][:, c, :] if c < 6 else f_[j][0:PLAST, c, :]
            for nh in range(NH):
                nc.tensor.matmul(
                    psum[nh][:], lhs, wslice(c, j, nh),
                    start=(nmm == 0), stop=(nmm == NTOT - 1))

        # ---- tail: h1 = tanh(y), transpose, blockdiag MLP, reduce ----
        h1 = sb.tile([128, HO], f16, tag="h1")
        SEG = [(0, 0, 128), (0, 128, 256), (0, 256, 320), (1, 320, 384),
               (1, 384, 512), (1, 512, 640)]

        def tanh_seg(k):
            nh, s0, s1 = SEG[k]
            nc.scalar.activation(h1[:, s0:s1],
                                 psum[nh][:, s0 - nh * D2:s1 - nh * D2],
                                 AF.Tanh)

        h1t = []

        def tr(k):
            pt = ps.tile([128, 128], f16, tag=f"pt{k}", name=f"pt{k}")
            nc.tensor.transpose(pt[:], h1[:, k * 128:(k + 1) * 128], idt)
            st = sb.tile([128, 128], f16, tag=f"h1t{k}", name=f"h1t{k}")
            nc.vector.tensor_copy(st[:], pt[:])
            h1t.append(st)

        tanh_seg(0); tr(0)
        tanh_seg(1); tr(1)
        tanh_seg(2); tanh_seg(3); tr(2)
        tanh_seg(4); tr(3)
        tanh_seg(5); tr(4)

        ps2 = ps.tile([128, D2], f32, tag="ps2")
        for k in range(5):
            nc.tensor.matmul(ps2[:], h1t[k][:], w1t[:, k, :],
                             start=(k == 0), stop=False)
        nc.tensor.matmul(ps2[:], ones[:], b1r[:], start=False, stop=True)
        h2 = sb.tile([128, D2], f32, tag="h2")
        nc.scalar.activation(h2[:], ps2[:], AF.Tanh)
        prod = sb.tile([128, D2], f32, tag="prod")
        nc.vector.tensor_tensor(prod[:], h2[:], w2b, op=ALU.mult)
        red = sb.tile([128, H], f32, tag="red")
        nc.vector.tensor_reduce(red[:], prod[:].rearrange("p (h d) -> p h d", d=32),
                                axis=mybir.AxisListType.X, op=ALU.add)
        lg = sb.tile([128, H], f32, tag="lg")
        nc.vector.tensor_tensor(lg[:], red[:], b2b, op=ALU.add)
        nc.sync.dma_start(out_d, lg[:])

    nc.compile()
    _CACHE["nc"] = nc
    return nc


def _prep_inputs(x, coef, scale_base, scale_sp, lmd, W1, b1, W2, b2):
    xf = np.asarray(x, np.float64).reshape(B, I)
    coef = np.asarray(coef, np.float64)
    eff = coef * np.asarray(scale_sp, np.float64)[..., None] \
        * np.asarray(lmd, np.float64)[:, :, None, None] / 6.0
    sbl = np.asarray(scale_base, np.float64) \
        * np.asarray(lmd, np.float64)[:, :, None]
    wbig = np.concatenate([eff, sbl[..., None]], -1)            # (H,I,O,9)
    # -> (I, 9, H, O), then piece-major per chunk
    wi = np.ascontiguousarray(wbig.transpose(1, 3, 0, 2)).astype(np.float16)
    pieces = [wi[c * 128:(c + 1) * 128].reshape(-1) for c in range(6)]
    pieces.append(np.ascontiguousarray(wi[768:I, :, 0:5, :]).reshape(-1))
    pieces.append(np.ascontiguousarray(wi[768:I, :, 5:10, :]).reshape(-1))
    wdev = np.concatenate(pieces)

    W1 = np.asarray(W1, np.float64)
    w1bd = np.zeros((HO, D2))
    for h in range(H):
        w1bd[h * O:(h + 1) * O, h * 32:(h + 1) * 32] = W1[h]
    w1dev = np.ascontiguousarray(
        w1bd.reshape(5, 128, D2).transpose(1, 0, 2)).astype(np.float16)
    c16 = np.concatenate([w1dev.reshape(128, 5 * D2),
                          np.eye(128, dtype=np.float16)], 1).astype(np.float16)
    b1c = np.asarray(b1, np.float16).reshape(1, D2).copy()
    c32 = np.ascontiguousarray(np.concatenate([
        np.broadcast_to(np.asarray(W2, np.float32).reshape(D2), (128, D2)),
        np.broadcast_to(np.asarray(b2, np.float32).reshape(H), (128, H))],
        1).astype(np.float32))

    in_maps = []
    for core in range(NC):
        xs = xf[core * BC:(core + 1) * BC].T                     # (784,128)
        xdev = np.zeros((128, CH, BC), np.float32)
        for c in range(CH):
            rows = xs[c * 128:min((c + 1) * 128, I)]
            xdev[0:rows.shape[0], c, :] = rows
        in_maps.append({"x": xdev, "w": wdev, "w1": c16,
                        "b1": b1c, "w2": c32})
    return in_maps


def run(inputs, trace=False, tmpdir=None):
    _install_ntff_hook()
    from concourse.bass_utils import run_bass_kernel_spmd
    nc = _build()
    in_maps = _prep_inputs(**inputs)
    res = run_bass_kernel_spmd(nc, in_maps, core_ids=list(range(NC)),
                               trace=trace, tmpdir=tmpdir)
    out = np.concatenate([r["out"] for r in res.results], 0)
    return out.astype(np.float32), res


def kernel(**inputs):
    out, _ = run(inputs)
    return out

